# revision 8
# baseline (speedup 1.0000x reference)
"""Causal self-attention (B=4, T=1024, C=2048, H=16, rotary) on 8 trn2 cores.

Sharding: core c = 2*b + g handles batch b, head-group g (heads 8g..8g+7).
 - QKV projection in transposed layout (Q^T/K^T = [d, T]; V natural [T, d]).
 - RoPE via full-height cos/sin tables; rotate-half partition swap runs on
   the PE as a permutation matmul.
 - Scores transposed S^T = K^T.Q -> [k, q]; softmax without max-subtraction;
   causal masking via multiplicative 0/1 bf16 masks on diagonal blocks.
 - Softmax denominator: ones[128,128] matmul accumulates column sums already
   broadcast across partitions; 1/d = exp(-ln d) on the scalar engine (Ln и
   Exp share one ACT table set, so no table reloads).
 - att@V accumulated in PSUM -> y^T [d, q], normalized by rb while copying
   PSUM->SBUF.
 - Emission interleaves phases so the Tile scheduler overlaps them:
   chunks(q0,k0,v0) -> attn h0-3 -> AllGather(h0-3) -> chunks(q1,k1,v1)
   -> attn h4-7 -> AllGather(h4-7) -> c_proj waveA (gathered heads 0-3 both
   cores, + bias -> stash) -> waveB (remaining heads) + merge -> out.
All matmuls bf16 (fp32 PSUM accumulation).
"""

import math

import numpy as np
import ml_dtypes

BF16 = ml_dtypes.bfloat16

B, T, C = 4, 1024, 2048
H = 16  # total heads
D = C // H  # 128 head dim
HG = 8  # heads per group (per core)
N_CORES = 8
ROPE_BASE = 10000.0

TUNE = {
    "ps_a": 2,
    "ps_b": 3,
    "ps_y": 2,
    "p_sb_bufs": 6,
}

_PROGRAM_CACHE = {}


def _build_program(num_devices=N_CORES, collective=True):
    import concourse.mybir as mybir
    import concourse.tile as tile
    from concourse import bacc
    from concourse.bass import ts

    f32 = mybir.dt.float32
    bf16 = mybir.dt.bfloat16
    AF = mybir.ActivationFunctionType

    nc = bacc.Bacc(trn_type="TRN2", num_devices=num_devices, debug=False)

    # ---- per-core I/O ----
    xT = nc.dram_tensor("xT", [C, T], bf16, kind="ExternalInput")  # x[b].T
    wqkv = nc.dram_tensor("wqkv", [C, 3 * HG * D], bf16, kind="ExternalInput")
    bqk = nc.dram_tensor("bqk", [128, 16], f32, kind="ExternalInput")
    bv = nc.dram_tensor("bv", [1, HG * D], f32, kind="ExternalInput")
    # full-height rope tables: cos2 = [cos; cos], sin2 = [-sin; sin]
    cosT = nc.dram_tensor("cosT", [D, T], bf16, kind="ExternalInput")
    sinT = nc.dram_tensor("sinT", [D, T], bf16, kind="ExternalInput")
    maskT = nc.dram_tensor("maskT", [128, 4, 512], bf16, kind="ExternalInput")
    # half-swap permutation: perm[j2, j] = 1 iff j2 == (j + 64) % 128
    perm = nc.dram_tensor("perm", [128, 128], bf16, kind="ExternalInput")
    wproj = nc.dram_tensor("wproj", [C, C // 2], bf16, kind="ExternalInput")
    bproj = nc.dram_tensor("bproj", [1, C // 2], f32, kind="ExternalInput")
    out = nc.dram_tensor("out", [T, C // 2], f32, kind="ExternalOutput")

    xT_r = xT.ap().rearrange("(ct p) t -> p ct t", p=128)  # [128, 16, 1024]
    wqkv_r = wqkv.ap().rearrange("(ct p) j -> p ct j", p=128)  # [128, 16, 3072]
    wproj_r = wproj.ap().rearrange("(jt p) c -> p jt c", p=128)  # [128, 16, 1024]

    scale = 1.0 / math.sqrt(D)

    with tile.TileContext(nc) as tc:
        with (
            tc.tile_pool(name="const", bufs=1) as const,
            tc.tile_pool(name="persist", bufs=1) as persist,
            tc.tile_pool(name="wp_pool", bufs=1) as wp_pool,
            tc.tile_pool(name="ps_a", bufs=TUNE["ps_a"], space="PSUM") as psA,
            tc.tile_pool(name="ps_b", bufs=TUNE["ps_b"], space="PSUM") as psB,
            tc.tile_pool(name="ps_y", bufs=TUNE["ps_y"], space="PSUM") as psY,
            tc.tile_pool(name="ps_sum", bufs=1, space="PSUM") as psS,
            tc.tile_pool(name="work", bufs=4) as work,
            tc.tile_pool(name="dram", bufs=1, space="DRAM") as drampool,
        ):
            # ---- persistent activations ----
            qf = persist.tile([128, HG, T], bf16)  # [d, h, t] rotated Q^T
            kf = persist.tile([128, HG, T], bf16)  # [d, h, t] rotated K^T
            v_all = persist.tile([128, 8, HG * D], bf16)  # [t_in, tt, j]
            yT = persist.tile([128, HG, T], bf16)  # [d, h, t] normalized att out

            # ---- DRAM staging for the collective ----
            # groups of local heads per AllGather: smaller groups at the end
            # so the tail AG (after the last head) is small and fast
            AG_HEADS = ((0, 1), (2, 3), (4, 5), (6,), (7,))
            ybounce = drampool.tile([HG * D, T], bf16, name="ybounce")
            ygth_q = [
                drampool.tile([2 * len(hs) * D, T], bf16, name=f"ygth{w}")
                for w, hs in enumerate(AG_HEADS)
            ]
            yb_r = ybounce.rearrange("(h p) t -> p h t", p=128)

            with (
                tc.tile_pool(name="xpool", bufs=1) as xpool,
                tc.tile_pool(name="wpool", bufs=2) as wpool,
            ):
                xs = xpool.tile([128, 16, T], bf16, name="xs")
                wts = {}

                def load_chunk(chunk, interleave_xs=False):
                    wt = wpool.tile([128, 16, 512], bf16, tag="wt", name="wt")
                    wts[chunk] = wt
                    cslice = slice(chunk * 512, (chunk + 1) * 512)
                    if interleave_xs:
                        # 2-ct granules: first matmul chain starts after ~1/8
                        # of the data, and DMA-issue count stays low
                        for q in range(8):
                            cts = slice(2 * q, 2 * q + 2)
                            nc.sync.dma_start(out=xs[:, cts, :], in_=xT_r[:, cts, :])
                            nc.sync.dma_start(
                                out=wt[:, cts, :], in_=wqkv_r[:, cts, cslice]
                            )
                    else:
                        nc.sync.dma_start(out=wt, in_=wqkv_r[:, :, cslice])

                # first chunk's weights interleaved with xs so matmuls start
                # within ~2us of kernel start
                load_chunk(0, interleave_xs=True)

                # ---- constants (after the critical first-chunk DMAs) ----
                bqk_sb = const.tile([128, 16], f32)
                nc.sync.dma_start(out=bqk_sb, in_=bqk.ap())
                cos_sb = const.tile([128, T], bf16)
                nc.sync.dma_start(out=cos_sb, in_=cosT.ap())
                sin_sb = const.tile([128, T], bf16)
                nc.sync.dma_start(out=sin_sb, in_=sinT.ap())
                perm_sb = const.tile([128, 128], bf16)
                nc.sync.dma_start(out=perm_sb, in_=perm.ap())
                ones128 = const.tile([128, 128], bf16)
                nc.vector.memset(ones128, 1.0)
                mask_sb = const.tile([128, 4, 512], bf16)
                nc.sync.dma_start(out=mask_sb, in_=maskT.ap())
                bv_bc = const.tile([128, HG * D], f32)
                nc.sync.dma_start(out=bv_bc, in_=bv.ap().to_broadcast([128, HG * D]))
                bp_bc = const.tile([128, C // 2], f32)
                nc.sync.dma_start(out=bp_bc, in_=bproj.ap().to_broadcast([128, C // 2]))

                wp = wp_pool.tile([128, 16, C // 2], bf16, name="wp")

                def emit_chunk(chunk):
                    wt = wts[chunk]
                    if chunk < 4:  # Q or K, output transposed [j, t]
                        for jj in range(4):
                            jt = chunk * 4 + jj  # q: 0-7, k: 8-15
                            h = jt % 8
                            dest_all = qf if jt < 8 else kf
                            for th in range(2):
                                ps = psA.tile([128, 512], f32, tag="ps", name="ps")
                                for ct in range(16):
                                    nc.tensor.matmul(
                                        ps,
                                        lhsT=wt[:, ct, jj * 128 : (jj + 1) * 128],
                                        rhs=xs[:, ct, ts(th, 512)],
                                        start=(ct == 0),
                                        stop=(ct == 15),
                                    )
                                raw = work.tile(
                                    [128, 512], bf16, tag="raw", name="raw",
                                    bufs=3,
                                )
                                # bias-add on ACT (Identity supports AP bias);
                                # keeps DVE free for the rope muls
                                nc.scalar.activation(
                                    raw, ps, AF.Identity,
                                    bias=bqk_sb[:, jt : jt + 1],
                                )
                                dest = dest_all[:, h, ts(th, 512)]
                                ps_swp = psB.tile(
                                    [128, 512], f32, tag="psb", name="ps_swp"
                                )
                                nc.tensor.matmul(
                                    ps_swp, lhsT=perm_sb, rhs=raw,
                                    start=True, stop=True,
                                )
                                rtmp = work.tile(
                                    [128, 512], bf16, tag="rtmp", name="rtmp",
                                    bufs=3,
                                )
                                nc.vector.tensor_mul(
                                    rtmp, ps_swp, sin_sb[:, ts(th, 512)]
                                )
                                nc.vector.tensor_mul(
                                    dest, raw, cos_sb[:, ts(th, 512)]
                                )
                                nc.vector.tensor_add(dest, dest, rtmp)
                    else:  # V, natural layout [t, j]
                        jc = chunk - 4  # 0 or 1
                        for tt in range(8):
                            ps = psA.tile([128, 512], f32, tag="ps", name="ps")
                            for ct in range(16):
                                nc.tensor.matmul(
                                    ps,
                                    lhsT=xs[:, ct, ts(tt, 128)],
                                    rhs=wt[:, ct, :],
                                    start=(ct == 0),
                                    stop=(ct == 15),
                                )
                            nc.vector.tensor_add(
                                v_all[:, tt, jc * 512 : (jc + 1) * 512],
                                ps,
                                bv_bc[:, jc * 512 : (jc + 1) * 512],
                            )

                def emit_attn(h):
                    for qc in range(2):
                        n_kt = 4 * (qc + 1)
                        ps_y = psY.tile([128, 512], f32, tag="ps_y", name="ps_y")
                        ps_sum = psS.tile(
                            [128, 512], f32, tag="ps_sum", name="ps_sum"
                        )
                        p_hold = None
                        for kt in range(n_kt):
                            ps_sc = psB.tile(
                                [128, 512], f32, tag="psb", name="ps_sc"
                            )
                            nc.tensor.matmul(
                                ps_sc,
                                lhsT=kf[:, h, ts(kt, 128)],
                                rhs=qf[:, h, ts(qc, 512)],
                                start=True,
                                stop=True,
                            )
                            p_sb = work.tile(
                                [128, 512], bf16, tag="p_sb", name="p_sb",
                                bufs=TUNE["p_sb_bufs"],
                            )
                            nc.scalar.activation(p_sb, ps_sc, AF.Exp, scale=scale)
                            kt_rel = kt - 4 * qc
                            if 0 <= kt_rel < 4:  # block straddles the diagonal
                                nc.vector.tensor_mul(
                                    p_sb, p_sb, mask_sb[:, kt_rel, :]
                                )
                            if kt % 2 == 0:
                                p_hold = p_sb
                            else:
                                padd = work.tile(
                                    [128, 512], bf16, tag="padd", name="padd",
                                    bufs=3,
                                )
                                nc.vector.tensor_add(padd, p_hold, p_sb)
                                if kt % 4 == 1:
                                    padd_hold = padd
                                else:
                                    pquad = work.tile(
                                        [128, 512], bf16, tag="pquad",
                                        name="pquad", bufs=2,
                                    )
                                    nc.vector.tensor_add(pquad, padd_hold, padd)
                                    # ones[128,128] stationary: column sums land
                                    # broadcast across all 128 partitions
                                    nc.tensor.matmul(
                                        ps_sum,
                                        lhsT=ones128,
                                        rhs=pquad,
                                        start=(kt == 3),
                                        stop=(kt == n_kt - 1),
                                    )
                            nc.tensor.matmul(
                                ps_y,
                                lhsT=v_all[:, kt, ts(h, 128)],
                                rhs=p_sb,
                                start=(kt == 0),
                                stop=(kt == n_kt - 1),
                            )
                        # 1/denom = exp(-ln(denom)); Ln/Exp share a table set
                        lnt = work.tile(
                            [128, 512], f32, tag="lnt", name="lnt", bufs=2
                        )
                        nc.scalar.activation(lnt, ps_sum, AF.Ln)
                        rb = work.tile(
                            [128, 512], bf16, tag="rb", name="rb", bufs=2
                        )
                        nc.scalar.activation(rb, lnt, AF.Exp, scale=-1.0)
                        nc.vector.tensor_mul(yT[:, h, ts(qc, 512)], ps_y, rb)
                    nc.sync.dma_start(out=yb_r[:, h, :], in_=yT[:, h, :])

                # ========== interleaved emission ==========
                def emit_ag(w):
                    hs = AG_HEADS[w]
                    rows = slice(D * hs[0], D * (hs[-1] + 1))
                    half = len(hs) * D
                    if collective:
                        nc.gpsimd.collective_compute(
                            "AllGather",
                            mybir.AluOpType.bypass,
                            replica_groups=[[0, 1], [2, 3], [4, 5], [6, 7]],
                            ins=[ybounce[rows, :].opt()],
                            outs=[ygth_q[w][:].opt()],
                        )
                    else:
                        nc.sync.dma_start(
                            out=ygth_q[w][0:half, :], in_=ybounce[rows, :]
                        )
                        nc.sync.dma_start(
                            out=ygth_q[w][half:, :], in_=ybounce[rows, :]
                        )

                ag_after = {1: 0, 3: 1, 5: 2, 6: 3, 7: 4}
                for chunk in (2, 4):
                    load_chunk(chunk)
                for chunk in (0, 2, 4):
                    emit_chunk(chunk)
                for h in range(4):
                    emit_attn(h)
                    if h in ag_after:
                        emit_ag(ag_after[h])
                for chunk in (1, 3, 5):
                    load_chunk(chunk)
                # proj weights: DMA issues behind the chunk loads, done long
                # before the first wave needs it
                nc.sync.dma_start(out=wp, in_=wproj_r)
                for chunk in (1, 3, 5):
                    emit_chunk(chunk)
                for h in range(4, 8):
                    emit_attn(h)
                    if h in ag_after:
                        emit_ag(ag_after[h])

            # ========== c_proj: waves over the gathered groups ==========
            # ygth_q[w] rows = AG_HEADS[w] from rank0 then rank1 -> global jt
            # = heads and heads+8. Wave 0 covers AG groups 0+1 (8 jt) to halve
            # the DVE merge count; later waves shrink so the post-h7 tail is
            # only a 2-matmul chain per tile.
            with (
                tc.tile_pool(name="stash_pool", bufs=1) as stash_pool,
                tc.tile_pool(name="ygs_pool", bufs=1) as ygs_pool,
            ):
                stash = stash_pool.tile([128, 16, 512], f32, name="stash")
                ygs_t = []
                for w, hs in enumerate(AG_HEADS):
                    nj = 2 * len(hs)
                    ygs = ygs_pool.tile(
                        [128, nj, T], bf16, tag=f"ygs{w}", name=f"ygs{w}"
                    )
                    nc.sync.dma_start(
                        out=ygs,
                        in_=ygth_q[w].rearrange("(j p) t -> p j t", p=128),
                    )
                    ygs_t.append(ygs)

                # (ygs tile, slot, global jt) per wave
                def group_ops(w):
                    hs = AG_HEADS[w]
                    ops = []
                    for r in range(2):  # rank 0: heads hs; rank 1: heads hs+8
                        for i, h in enumerate(hs):
                            ops.append(
                                (ygs_t[w], r * len(hs) + i, 8 * r + h)
                            )
                    return ops

                waves = [group_ops(0) + group_ops(1)] + [
                    group_ops(w) for w in (2, 3, 4)
                ]
                for wv, ops in enumerate(waves):
                    for tt in range(8):
                        for cc in range(2):
                            st = tt * 2 + cc
                            ps = psA.tile(
                                [128, 512], f32, tag="ps", name="ps_proj"
                            )
                            for i, (ygs, slot, jt) in enumerate(ops):
                                nc.tensor.matmul(
                                    ps,
                                    lhsT=ygs[:, slot, ts(tt, 128)],
                                    rhs=wp[:, jt, ts(cc, 512)],
                                    start=(i == 0),
                                    stop=(i == len(ops) - 1),
                                )
                            if wv == 0:
                                nc.vector.tensor_add(
                                    stash[:, st, :], ps, bp_bc[:, ts(cc, 512)]
                                )
                            elif wv < len(waves) - 1:
                                nc.vector.tensor_add(
                                    stash[:, st, :], ps, stash[:, st, :]
                                )
                            else:
                                o_sb = work.tile(
                                    [128, 512], f32, tag="o_sb", name="o_sb",
                                    bufs=3,
                                )
                                nc.vector.tensor_add(o_sb, ps, stash[:, st, :])
                                nc.sync.dma_start(
                                    out=out.ap()[ts(tt, 128), ts(cc, 512)],
                                    in_=o_sb,
                                )

    # Pin every activation to the one table set holding Exp+Ln+Identity
    # (index 6, natural_log_exp_and_others). Без этого the set-picker
    # alternates exp_and_others <-> natural_log per instruction and inserts
    # ~33 ACT_TABLE_LOADs (~50us of Scalar-engine time).
    import concourse.bacc as bacc_mod

    orig_tables = bacc_mod.get_activation_tables

    def _pinned_tables(arch):
        tabs = orig_tables(arch)
        return {
            name: (funcs if name == "natural_log_exp_and_others" else set())
            for name, funcs in tabs.items()
        }

    bacc_mod.get_activation_tables = _pinned_tables
    try:
        nc.finalize()
    finally:
        bacc_mod.get_activation_tables = orig_tables
    return nc


def _host_inputs(x, w_attn, b_attn, w_proj, b_proj):
    """Build the 8 per-core input maps."""
    x = np.asarray(x, np.float32)
    w_attn = np.asarray(w_attn, np.float32)
    b_attn = np.asarray(b_attn, np.float32)
    w_proj = np.asarray(w_proj, np.float32)
    b_proj = np.asarray(b_proj, np.float32)

    # rope tables, transposed [d, t], full height with rotate-half signs folded:
    # out = x * cos2 + swap_halves(x) * sin2,  cos2=[cos;cos], sin2=[-sin;sin]
    inv_freq = 1.0 / (ROPE_BASE ** (np.arange(0, D, 2, dtype=np.float32) / D))
    freqs = np.arange(T, dtype=np.float32)[:, None] * inv_freq[None, :]  # [T, 64]
    c_ = np.ascontiguousarray(np.cos(freqs).T)  # [64, T]
    s_ = np.ascontiguousarray(np.sin(freqs).T)
    cosT = np.concatenate([c_, c_], axis=0).astype(BF16)  # [128, T]
    sinT = np.concatenate([-s_, s_], axis=0).astype(BF16)

    # causal mask blocks, transposed [k, q]: block kt_rel r, q chunk of 512
    k_idx = np.arange(128)
    q_idx = np.arange(512)
    maskT = np.zeros((128, 4, 512), np.float32)
    for r in range(4):
        maskT[:, r, :] = ((r * 128 + k_idx)[:, None] <= q_idx[None, :]).astype(
            np.float32
        )
    maskT = maskT.astype(BF16)

    permM = np.zeros((128, 128), np.float32)
    permM[(np.arange(128) + 64) % 128, np.arange(128)] = 1.0
    permM = permM.astype(BF16)

    in_maps = []
    for c in range(N_CORES):
        b, g = divmod(c, 2)
        cs = slice(g * 1024, (g + 1) * 1024)
        wq = w_attn[:, 0:C][:, cs]
        wk = w_attn[:, C : 2 * C][:, cs]
        wv = w_attn[:, 2 * C : 3 * C][:, cs]
        bq = b_attn[0:C][cs]
        bk = b_attn[C : 2 * C][cs]
        bvv = b_attn[2 * C : 3 * C][cs]
        in_maps.append(
            {
                "xT": np.ascontiguousarray(x[b].T).astype(BF16),
                "wqkv": np.concatenate([wq, wk, wv], axis=1).astype(BF16),
                "bqk": np.ascontiguousarray(
                    np.concatenate([bq, bk]).reshape(16, 128).T
                ).astype(np.float32),
                "bv": bvv.reshape(1, 1024).astype(np.float32),
                "cosT": cosT,
                "sinT": sinT,
                "maskT": maskT,
                "perm": permM,
                "wproj": w_proj[:, cs].astype(BF16),
                "bproj": b_proj[cs].reshape(1, 1024).astype(np.float32),
            }
        )
    return in_maps


def kernel(x, w_attn, b_attn, w_proj, b_proj, _trace=False):
    from concourse.bass_utils import run_bass_kernel_spmd

    if "nc" not in _PROGRAM_CACHE:
        _PROGRAM_CACHE["nc"] = _build_program()
    nc = _PROGRAM_CACHE["nc"]

    in_maps = _host_inputs(x, w_attn, b_attn, w_proj, b_proj)
    res = run_bass_kernel_spmd(
        nc, in_maps, core_ids=list(range(N_CORES)), trace=_trace
    )
    _PROGRAM_CACHE["last_results"] = res

    out = np.zeros((B, T, C), np.float32)
    for c in range(N_CORES):
        b, g = divmod(c, 2)
        out[b, :, g * 1024 : (g + 1) * 1024] = res.results[c]["out"]
    return out


# revision 9
# speedup vs baseline: 1.0526x; 1.0526x over previous
"""Causal self-attention (B=4, T=1024, C=2048, H=16, rotary) on 8 trn2 cores.

Sharding: core c = 2*b + g handles batch b, head-group g (heads 8g..8g+7).
 - QKV projection in transposed layout (Q^T/K^T = [d, T]; V natural [T, d]).
 - RoPE via full-height cos/sin tables; rotate-half partition swap runs on
   the PE as a permutation matmul.
 - Scores transposed S^T = K^T.Q -> [k, q]; softmax without max-subtraction;
   causal masking via multiplicative 0/1 bf16 masks on diagonal blocks.
 - Softmax denominator: ones[128,128] matmul accumulates column sums already
   broadcast across partitions; 1/d = exp(-ln d) on the scalar engine (Ln и
   Exp share one ACT table set, so no table reloads).
 - att@V accumulated in PSUM -> y^T [d, q], normalized by rb while copying
   PSUM->SBUF.
 - Emission interleaves phases so the Tile scheduler overlaps them:
   chunks(q0,k0,v0) -> attn h0-3 -> AllGather(h0-3) -> chunks(q1,k1,v1)
   -> attn h4-7 -> AllGather(h4-7) -> c_proj waveA (gathered heads 0-3 both
   cores, + bias -> stash) -> waveB (remaining heads) + merge -> out.
All matmuls bf16 (fp32 PSUM accumulation).
"""

import math

import numpy as np
import ml_dtypes

BF16 = ml_dtypes.bfloat16

B, T, C = 4, 1024, 2048
H = 16  # total heads
D = C // H  # 128 head dim
HG = 8  # heads per group (per core)
N_CORES = 8
ROPE_BASE = 10000.0

TUNE = {
    "ps_a": 2,
    "ps_b": 3,
    "ps_y": 2,
    "p_sb_bufs": 6,
}

_PROGRAM_CACHE = {}


def _build_program(num_devices=N_CORES, collective=True):
    import concourse.mybir as mybir
    import concourse.tile as tile
    from concourse import bacc
    from concourse.bass import ts

    f32 = mybir.dt.float32
    bf16 = mybir.dt.bfloat16
    AF = mybir.ActivationFunctionType

    nc = bacc.Bacc(trn_type="TRN2", num_devices=num_devices, debug=False)

    # ---- per-core I/O ----
    xT = nc.dram_tensor("xT", [C, T], bf16, kind="ExternalInput")  # x[b].T
    wqkv = nc.dram_tensor("wqkv", [C, 3 * HG * D], bf16, kind="ExternalInput")
    bqk = nc.dram_tensor("bqk", [128, 16], f32, kind="ExternalInput")
    bv = nc.dram_tensor("bv", [1, HG * D], f32, kind="ExternalInput")
    # full-height rope tables: cos2 = [cos; cos], sin2 = [-sin; sin]
    cosT = nc.dram_tensor("cosT", [D, T], bf16, kind="ExternalInput")
    sinT = nc.dram_tensor("sinT", [D, T], bf16, kind="ExternalInput")
    maskT = nc.dram_tensor("maskT", [128, 4, 512], bf16, kind="ExternalInput")
    # half-swap permutation: perm[j2, j] = 1 iff j2 == (j + 64) % 128
    perm = nc.dram_tensor("perm", [128, 128], bf16, kind="ExternalInput")
    wproj = nc.dram_tensor("wproj", [C, C // 2], bf16, kind="ExternalInput")
    bproj = nc.dram_tensor("bproj", [1, C // 2], f32, kind="ExternalInput")
    out = nc.dram_tensor("out", [T, C // 2], f32, kind="ExternalOutput")

    xT_r = xT.ap().rearrange("(ct p) t -> p ct t", p=128)  # [128, 16, 1024]
    wqkv_r = wqkv.ap().rearrange("(ct p) j -> p ct j", p=128)  # [128, 16, 3072]
    wproj_r = wproj.ap().rearrange("(jt p) c -> p jt c", p=128)  # [128, 16, 1024]

    scale = 1.0 / math.sqrt(D)

    with tile.TileContext(nc) as tc:
        with (
            tc.tile_pool(name="const", bufs=1) as const,
            tc.tile_pool(name="persist", bufs=1) as persist,
            tc.tile_pool(name="wp_pool", bufs=1) as wp_pool,
            tc.tile_pool(name="ps_a", bufs=TUNE["ps_a"], space="PSUM") as psA,
            tc.tile_pool(name="ps_b", bufs=TUNE["ps_b"], space="PSUM") as psB,
            tc.tile_pool(name="ps_y", bufs=TUNE["ps_y"], space="PSUM") as psY,
            tc.tile_pool(name="ps_sum", bufs=1, space="PSUM") as psS,
            tc.tile_pool(name="work", bufs=4) as work,
            tc.tile_pool(name="dram", bufs=1, space="DRAM") as drampool,
        ):
            # ---- persistent activations ----
            qf = persist.tile([128, HG, T], bf16)  # [d, h, t] rotated Q^T
            kf = persist.tile([128, HG, T], bf16)  # [d, h, t] rotated K^T
            v_all = persist.tile([128, 8, HG * D], bf16)  # [t_in, tt, j]
            yT = persist.tile([128, HG, T], bf16)  # [d, h, t] normalized att out

            # ---- DRAM staging for the collective ----
            # groups of local heads per AllGather: smaller groups at the end
            # so the tail AG (after the last head) is small and fast
            AG_HEADS = ((0, 1), (2, 3), (4, 5), (6, 7))
            ybounce = drampool.tile([HG * D, T], bf16, name="ybounce")
            ygth_q = [
                drampool.tile([2 * len(hs) * D, T], bf16, name=f"ygth{w}")
                for w, hs in enumerate(AG_HEADS)
            ]
            yb_r = ybounce.rearrange("(h p) t -> p h t", p=128)

            with (
                tc.tile_pool(name="xpool", bufs=1) as xpool,
                tc.tile_pool(name="wpool", bufs=2) as wpool,
            ):
                xs = xpool.tile([128, 16, T], bf16, name="xs")
                wts = {}

                def load_chunk(chunk, interleave_xs=False):
                    wt = wpool.tile([128, 16, 512], bf16, tag="wt", name="wt")
                    wts[chunk] = wt
                    cslice = slice(chunk * 512, (chunk + 1) * 512)
                    if interleave_xs:
                        # 2-ct granules: first matmul chain starts after ~1/8
                        # of the data, and DMA-issue count stays low
                        for q in range(8):
                            cts = slice(2 * q, 2 * q + 2)
                            nc.sync.dma_start(out=xs[:, cts, :], in_=xT_r[:, cts, :])
                            nc.sync.dma_start(
                                out=wt[:, cts, :], in_=wqkv_r[:, cts, cslice]
                            )
                    else:
                        nc.sync.dma_start(out=wt, in_=wqkv_r[:, :, cslice])

                # first chunk's weights interleaved with xs so matmuls start
                # within ~2us of kernel start
                load_chunk(0, interleave_xs=True)

                # ---- constants (after the critical first-chunk DMAs) ----
                bqk_sb = const.tile([128, 16], f32)
                nc.sync.dma_start(out=bqk_sb, in_=bqk.ap())
                cos_sb = const.tile([128, T], bf16)
                nc.sync.dma_start(out=cos_sb, in_=cosT.ap())
                sin_sb = const.tile([128, T], bf16)
                nc.sync.dma_start(out=sin_sb, in_=sinT.ap())
                perm_sb = const.tile([128, 128], bf16)
                nc.sync.dma_start(out=perm_sb, in_=perm.ap())
                ones128 = const.tile([128, 128], bf16)
                nc.vector.memset(ones128, 1.0)
                mask_sb = const.tile([128, 4, 512], bf16)
                nc.sync.dma_start(out=mask_sb, in_=maskT.ap())
                bv_bc = const.tile([128, HG * D], f32)
                nc.sync.dma_start(out=bv_bc, in_=bv.ap().to_broadcast([128, HG * D]))
                bp_bc = const.tile([128, C // 2], f32)
                nc.sync.dma_start(out=bp_bc, in_=bproj.ap().to_broadcast([128, C // 2]))

                wp = wp_pool.tile([128, 16, C // 2], bf16, name="wp")

                def emit_chunk(chunk):
                    wt = wts[chunk]
                    if chunk < 4:  # Q or K, output transposed [j, t]
                        for jj in range(4):
                            jt = chunk * 4 + jj  # q: 0-7, k: 8-15
                            h = jt % 8
                            dest_all = qf if jt < 8 else kf
                            for th in range(2):
                                ps = psA.tile([128, 512], f32, tag="ps", name="ps")
                                for ct in range(16):
                                    nc.tensor.matmul(
                                        ps,
                                        lhsT=wt[:, ct, jj * 128 : (jj + 1) * 128],
                                        rhs=xs[:, ct, ts(th, 512)],
                                        start=(ct == 0),
                                        stop=(ct == 15),
                                    )
                                raw = work.tile(
                                    [128, 512], bf16, tag="raw", name="raw",
                                    bufs=3,
                                )
                                # bias-add on ACT (Identity supports AP bias);
                                # keeps DVE free for the rope muls
                                nc.scalar.activation(
                                    raw, ps, AF.Identity,
                                    bias=bqk_sb[:, jt : jt + 1],
                                )
                                dest = dest_all[:, h, ts(th, 512)]
                                ps_swp = psB.tile(
                                    [128, 512], f32, tag="psb", name="ps_swp"
                                )
                                nc.tensor.matmul(
                                    ps_swp, lhsT=perm_sb, rhs=raw,
                                    start=True, stop=True,
                                )
                                rtmp = work.tile(
                                    [128, 512], bf16, tag="rtmp", name="rtmp",
                                    bufs=3,
                                )
                                nc.vector.tensor_mul(
                                    rtmp, ps_swp, sin_sb[:, ts(th, 512)]
                                )
                                nc.vector.tensor_mul(
                                    dest, raw, cos_sb[:, ts(th, 512)]
                                )
                                nc.vector.tensor_add(dest, dest, rtmp)
                    else:  # V, natural layout [t, j]
                        jc = chunk - 4  # 0 or 1
                        for tt in range(8):
                            ps = psA.tile([128, 512], f32, tag="ps", name="ps")
                            for ct in range(16):
                                nc.tensor.matmul(
                                    ps,
                                    lhsT=xs[:, ct, ts(tt, 128)],
                                    rhs=wt[:, ct, :],
                                    start=(ct == 0),
                                    stop=(ct == 15),
                                )
                            nc.vector.tensor_add(
                                v_all[:, tt, jc * 512 : (jc + 1) * 512],
                                ps,
                                bv_bc[:, jc * 512 : (jc + 1) * 512],
                            )

                def emit_attn(h):
                    for qc in range(2):
                        n_kt = 4 * (qc + 1)
                        ps_y = psY.tile([128, 512], f32, tag="ps_y", name="ps_y")
                        ps_sum = psS.tile(
                            [128, 512], f32, tag="ps_sum", name="ps_sum"
                        )
                        p_hold = None
                        for kt in range(n_kt):
                            ps_sc = psB.tile(
                                [128, 512], f32, tag="psb", name="ps_sc"
                            )
                            nc.tensor.matmul(
                                ps_sc,
                                lhsT=kf[:, h, ts(kt, 128)],
                                rhs=qf[:, h, ts(qc, 512)],
                                start=True,
                                stop=True,
                            )
                            p_sb = work.tile(
                                [128, 512], bf16, tag="p_sb", name="p_sb",
                                bufs=TUNE["p_sb_bufs"],
                            )
                            nc.scalar.activation(p_sb, ps_sc, AF.Exp, scale=scale)
                            kt_rel = kt - 4 * qc
                            if 0 <= kt_rel < 4:  # block straddles the diagonal
                                nc.vector.tensor_mul(
                                    p_sb, p_sb, mask_sb[:, kt_rel, :]
                                )
                            if kt % 2 == 0:
                                p_hold = p_sb
                            else:
                                padd = work.tile(
                                    [128, 512], bf16, tag="padd", name="padd",
                                    bufs=3,
                                )
                                nc.vector.tensor_add(padd, p_hold, p_sb)
                                if kt % 4 == 1:
                                    padd_hold = padd
                                else:
                                    pquad = work.tile(
                                        [128, 512], bf16, tag="pquad",
                                        name="pquad", bufs=2,
                                    )
                                    nc.vector.tensor_add(pquad, padd_hold, padd)
                                    # ones[128,128] stationary: column sums land
                                    # broadcast across all 128 partitions
                                    nc.tensor.matmul(
                                        ps_sum,
                                        lhsT=ones128,
                                        rhs=pquad,
                                        start=(kt == 3),
                                        stop=(kt == n_kt - 1),
                                    )
                            nc.tensor.matmul(
                                ps_y,
                                lhsT=v_all[:, kt, ts(h, 128)],
                                rhs=p_sb,
                                start=(kt == 0),
                                stop=(kt == n_kt - 1),
                            )
                        # 1/denom = exp(-ln(denom)); Ln/Exp share a table set
                        lnt = work.tile(
                            [128, 512], f32, tag="lnt", name="lnt", bufs=2
                        )
                        nc.scalar.activation(lnt, ps_sum, AF.Ln)
                        rb = work.tile(
                            [128, 512], bf16, tag="rb", name="rb", bufs=2
                        )
                        nc.scalar.activation(rb, lnt, AF.Exp, scale=-1.0)
                        nc.vector.tensor_mul(yT[:, h, ts(qc, 512)], ps_y, rb)
                    nc.sync.dma_start(out=yb_r[:, h, :], in_=yT[:, h, :])

                # ========== interleaved emission ==========
                def emit_ag(w):
                    hs = AG_HEADS[w]
                    rows = slice(D * hs[0], D * (hs[-1] + 1))
                    half = len(hs) * D
                    if collective:
                        nc.gpsimd.collective_compute(
                            "AllGather",
                            mybir.AluOpType.bypass,
                            replica_groups=[[0, 1], [2, 3], [4, 5], [6, 7]],
                            ins=[ybounce[rows, :].opt()],
                            outs=[ygth_q[w][:].opt()],
                        )
                    else:
                        nc.sync.dma_start(
                            out=ygth_q[w][0:half, :], in_=ybounce[rows, :]
                        )
                        nc.sync.dma_start(
                            out=ygth_q[w][half:, :], in_=ybounce[rows, :]
                        )

                ag_after = {1: 0, 3: 1, 5: 2, 7: 3}
                for chunk in (2, 4):
                    load_chunk(chunk)
                for chunk in (0, 2, 4):
                    emit_chunk(chunk)
                for h in range(4):
                    emit_attn(h)
                    if h in ag_after:
                        emit_ag(ag_after[h])
                for chunk in (1, 3, 5):
                    load_chunk(chunk)
                # proj weights: DMA issues behind the chunk loads, done long
                # before the first wave needs it
                nc.sync.dma_start(out=wp, in_=wproj_r)
                for chunk in (1, 3, 5):
                    emit_chunk(chunk)
                for h in range(4, 8):
                    emit_attn(h)
                    if h in ag_after:
                        emit_ag(ag_after[h])

            # ========== c_proj: waves over the gathered groups ==========
            # ygth_q[w] rows = AG_HEADS[w] from rank0 then rank1 -> global jt
            # = heads and heads+8. Wave 0 covers AG groups 0+1 (8 jt) to halve
            # the DVE merge count; later waves shrink so the post-h7 tail is
            # only a 2-matmul chain per tile.
            with (
                tc.tile_pool(name="stash_pool", bufs=1) as stash_pool,
                tc.tile_pool(name="ygs_pool", bufs=1) as ygs_pool,
            ):
                stash = stash_pool.tile([128, 16, 512], f32, name="stash")
                ygs_t = []
                for w, hs in enumerate(AG_HEADS):
                    nj = 2 * len(hs)
                    ygs = ygs_pool.tile(
                        [128, nj, T], bf16, tag=f"ygs{w}", name=f"ygs{w}"
                    )
                    nc.sync.dma_start(
                        out=ygs,
                        in_=ygth_q[w].rearrange("(j p) t -> p j t", p=128),
                    )
                    ygs_t.append(ygs)

                # (ygs tile, slot, global jt) per wave
                def group_ops(w):
                    hs = AG_HEADS[w]
                    ops = []
                    for r in range(2):  # rank 0: heads hs; rank 1: heads hs+8
                        for i, h in enumerate(hs):
                            ops.append(
                                (ygs_t[w], r * len(hs) + i, 8 * r + h)
                            )
                    return ops

                waves = [group_ops(0) + group_ops(1)] + [
                    group_ops(w) for w in (2, 3)
                ]
                for wv, ops in enumerate(waves):
                    for tt in range(8):
                        for cc in range(2):
                            st = tt * 2 + cc
                            ps = psA.tile(
                                [128, 512], f32, tag="ps", name="ps_proj"
                            )
                            for i, (ygs, slot, jt) in enumerate(ops):
                                nc.tensor.matmul(
                                    ps,
                                    lhsT=ygs[:, slot, ts(tt, 128)],
                                    rhs=wp[:, jt, ts(cc, 512)],
                                    start=(i == 0),
                                    stop=(i == len(ops) - 1),
                                )
                            if wv == 0:
                                nc.vector.tensor_add(
                                    stash[:, st, :], ps, bp_bc[:, ts(cc, 512)]
                                )
                            elif wv < len(waves) - 1:
                                nc.vector.tensor_add(
                                    stash[:, st, :], ps, stash[:, st, :]
                                )
                            else:
                                o_sb = work.tile(
                                    [128, 512], f32, tag="o_sb", name="o_sb",
                                    bufs=3,
                                )
                                nc.vector.tensor_add(o_sb, ps, stash[:, st, :])
                                nc.sync.dma_start(
                                    out=out.ap()[ts(tt, 128), ts(cc, 512)],
                                    in_=o_sb,
                                )

    # Pin every activation to the one table set holding Exp+Ln+Identity
    # (index 6, natural_log_exp_and_others). Без этого the set-picker
    # alternates exp_and_others <-> natural_log per instruction and inserts
    # ~33 ACT_TABLE_LOADs (~50us of Scalar-engine time).
    import concourse.bacc as bacc_mod

    orig_tables = bacc_mod.get_activation_tables

    def _pinned_tables(arch):
        tabs = orig_tables(arch)
        return {
            name: (funcs if name == "natural_log_exp_and_others" else set())
            for name, funcs in tabs.items()
        }

    bacc_mod.get_activation_tables = _pinned_tables
    try:
        nc.finalize()
    finally:
        bacc_mod.get_activation_tables = orig_tables
    return nc


def _host_inputs(x, w_attn, b_attn, w_proj, b_proj):
    """Build the 8 per-core input maps."""
    x = np.asarray(x, np.float32)
    w_attn = np.asarray(w_attn, np.float32)
    b_attn = np.asarray(b_attn, np.float32)
    w_proj = np.asarray(w_proj, np.float32)
    b_proj = np.asarray(b_proj, np.float32)

    # rope tables, transposed [d, t], full height with rotate-half signs folded:
    # out = x * cos2 + swap_halves(x) * sin2,  cos2=[cos;cos], sin2=[-sin;sin]
    inv_freq = 1.0 / (ROPE_BASE ** (np.arange(0, D, 2, dtype=np.float32) / D))
    freqs = np.arange(T, dtype=np.float32)[:, None] * inv_freq[None, :]  # [T, 64]
    c_ = np.ascontiguousarray(np.cos(freqs).T)  # [64, T]
    s_ = np.ascontiguousarray(np.sin(freqs).T)
    cosT = np.concatenate([c_, c_], axis=0).astype(BF16)  # [128, T]
    sinT = np.concatenate([-s_, s_], axis=0).astype(BF16)

    # causal mask blocks, transposed [k, q]: block kt_rel r, q chunk of 512
    k_idx = np.arange(128)
    q_idx = np.arange(512)
    maskT = np.zeros((128, 4, 512), np.float32)
    for r in range(4):
        maskT[:, r, :] = ((r * 128 + k_idx)[:, None] <= q_idx[None, :]).astype(
            np.float32
        )
    maskT = maskT.astype(BF16)

    permM = np.zeros((128, 128), np.float32)
    permM[(np.arange(128) + 64) % 128, np.arange(128)] = 1.0
    permM = permM.astype(BF16)

    in_maps = []
    for c in range(N_CORES):
        b, g = divmod(c, 2)
        cs = slice(g * 1024, (g + 1) * 1024)
        wq = w_attn[:, 0:C][:, cs]
        wk = w_attn[:, C : 2 * C][:, cs]
        wv = w_attn[:, 2 * C : 3 * C][:, cs]
        bq = b_attn[0:C][cs]
        bk = b_attn[C : 2 * C][cs]
        bvv = b_attn[2 * C : 3 * C][cs]
        in_maps.append(
            {
                "xT": np.ascontiguousarray(x[b].T).astype(BF16),
                "wqkv": np.concatenate([wq, wk, wv], axis=1).astype(BF16),
                "bqk": np.ascontiguousarray(
                    np.concatenate([bq, bk]).reshape(16, 128).T
                ).astype(np.float32),
                "bv": bvv.reshape(1, 1024).astype(np.float32),
                "cosT": cosT,
                "sinT": sinT,
                "maskT": maskT,
                "perm": permM,
                "wproj": w_proj[:, cs].astype(BF16),
                "bproj": b_proj[cs].reshape(1, 1024).astype(np.float32),
            }
        )
    return in_maps


def kernel(x, w_attn, b_attn, w_proj, b_proj, _trace=False):
    from concourse.bass_utils import run_bass_kernel_spmd

    if "nc" not in _PROGRAM_CACHE:
        _PROGRAM_CACHE["nc"] = _build_program()
    nc = _PROGRAM_CACHE["nc"]

    in_maps = _host_inputs(x, w_attn, b_attn, w_proj, b_proj)
    res = run_bass_kernel_spmd(
        nc, in_maps, core_ids=list(range(N_CORES)), trace=_trace
    )
    _PROGRAM_CACHE["last_results"] = res

    out = np.zeros((B, T, C), np.float32)
    for c in range(N_CORES):
        b, g = divmod(c, 2)
        out[b, :, g * 1024 : (g + 1) * 1024] = res.results[c]["out"]
    return out
